# revision 25
# baseline (speedup 1.0000x reference)
"""DeepTermRankingListNet Trainium2 kernel.

Full-input contract: kernel(**inputs) takes the unsharded numpy inputs and
returns the full [1, 256] output. Internally shards candidates C=256 across
8 NeuronCores (32 each), replicates the embedding table + small params,
runs one SPMD Bass/Tile kernel via run_bass_kernel_spmd, and concatenates
the per-core [32] outputs.

Per-core device program (CC=32 candidates, K=64 ctx rows, d=128):
  1. The embedding table is shipped to DRAM as bf16 (rel-err budget 2e-2;
     measured end-to-end error of the bf16 pipeline is ~3e-3, and the
     gathers move half the bytes). 17 indirect DMAs (one index per
     partition each -- the HW SWDGE ucode generates exactly one
     descriptor per offset-AP partition, so 128 rows per instruction is
     the hard limit) gather the 32*64 t2-ctx rows + 64 t1-ctx rows.
     The ~1.33us/instruction GpSimd issue cost dominates the kernel; all
     compute is arranged to chase the gather stream.
  2. Per pair-block: one bf16 PE transpose (single pass vs 2 for f32)
     into PSUM, copied to the chunk's BT tile.
  3. Per chunk of 4 pair-blocks: one k-major sim matmul with stationary
     AMT (bf16); tanh (bf16 out); rows-softmax numerators via a free-axis
     reduce; cols-softmax numerators via ones-vector matmuls on the tanh
     blocks (partition reduce on the PE, replacing the m-major sim
     matmuls + second tanh of the f32 version); exp writes straight into
     the LT checkerboard / ET2; newB^T per pair into PSUM column slices;
     bilinear partials (T1uT = (A@W)^T E chunk, PZ = T1uT * VBT chunk).
     Softmax max-subtraction is skipped: tanh output is in [-1,1] so exp
     never overflows.
  4. Softmax denominators via ones matmuls; z = ones-reduce of PZ;
     con = z/(s1*s2).
  5. Cosine-similarity string branch on [32, 200] f32 tiles entirely on
     DVE, with rsqrt via the quake bit-hack + 2 Newton steps so the
     scalar engine's activation-table set stays {Tanh, Exp} (a Sqrt
     would force two extra ~1.5us table loads + drains mid-kernel).
"""

import numpy as np

V, D, K, C, DS = 500000, 128, 64, 256, 200
NCORES = 8
CC = C // NCORES  # 32 candidates per core
NP = CC // 2      # 16 candidate-pair blocks
NB = NP + 1       # + 1 block for A (t1_ctx rows)
GAMMA = 0.5
RSQRT_MAGIC = 0x5F3759DF

_BUILT = None


def _build_nc():
    import concourse.bacc as bacc
    import concourse.mybir as mybir
    from concourse import bass
    from concourse.tile import TileContext

    f32 = mybir.dt.float32
    bf16 = mybir.dt.bfloat16
    i32 = mybir.dt.int32
    AF = mybir.ActivationFunctionType
    ALU = mybir.AluOpType
    AX = mybir.AxisListType

    nc = bacc.Bacc("TRN2", debug=False)

    table_d = nc.dram_tensor("table", (V, D), bf16, kind="ExternalInput")
    idx_d = nc.dram_tensor("idx", (128, NB), i32, kind="ExternalInput")
    # packed bf16 params: att | w | identity
    pb_d = nc.dram_tensor("pb", (128, 3 * D), bf16, kind="ExternalInput")
    # packed f32 params: str1(broadcast) | str2 | b
    ps_d = nc.dram_tensor("ps", (CC, 2 * DS + 1 + CC), f32, kind="ExternalInput")
    y_d = nc.dram_tensor("y", (1, CC), f32, kind="ExternalOutput")

    with TileContext(nc) as tc:
        with (
            tc.tile_pool(name="pers", bufs=1) as pp,
            tc.tile_pool(name="bts", bufs=2) as btp,
            tc.tile_pool(name="tanhk", bufs=2) as tkp,
            tc.tile_pool(name="ps_bt", bufs=2, space="PSUM") as ps_bt,
            tc.tile_pool(name="ps_simk", bufs=2, space="PSUM") as ps_simk,
            tc.tile_pool(name="ps_ck", bufs=2, space="PSUM") as ps_ck,
            tc.tile_pool(name="ps_sm", bufs=2, space="PSUM") as ps_sm,
        ):
            # ---- persistent SBUF tiles ----
            idx_sb = pp.tile([128, NB], i32, tag="idx")
            BGt = [
                pp.tile([128, D], bf16, tag=f"bg{j}", name=f"bg{j}")
                for j in range(NB)
            ]
            pb_sb = pp.tile([128, 3 * D], bf16, tag="pb")
            ps_sb = pp.tile([CC, 2 * DS + 1 + CC], f32, tag="ps")
            ones_bf = pp.tile([128, 1], bf16, tag="ones")
            A_T_sb = pp.tile([128, K], bf16, tag="at")
            AMT_sb = pp.tile([128, K], bf16, tag="amt")
            AW_sb = pp.tile([K, 128], bf16, tag="aw")
            LT = pp.tile([128, CC], bf16, tag="lt")
            R_T = pp.tile([K, CC], f32, tag="rt")
            ET2 = pp.tile([K, CC], bf16, tag="et2")
            VBT_sb = pp.tile([128, CC], f32, tag="vbt")
            PZ_sb = pp.tile([128, CC], bf16, tag="pz")
            scr200 = pp.tile([CC, DS], f32, tag="scr200")
            # small [CC,1] scalars (string branch + final combine)
            magic_sb = pp.tile([CC, 1], i32, tag="magic")
            dot_sb = pp.tile([CC, 1], f32, tag="dot")
            ssq2_sb = pp.tile([CC, 1], f32, tag="ssq2")
            ssq1_sb = pp.tile([CC, 1], f32, tag="ssq1")
            den2_sb = pp.tile([CC, 1], f32, tag="den2")
            rs_sb = pp.tile([CC, 1], f32, tag="rs")
            nt_sb = pp.tile([CC, 1], f32, tag="nt")
            strs_sb = pp.tile([CC, 1], f32, tag="strs")
            sbh_sb = pp.tile([CC, 1], f32, tag="sbh")
            s12_sb = pp.tile([CC, 1], f32, tag="s12")
            s2_sb2 = pp.tile([CC, 1], f32, tag="s2c")
            r12_sb = pp.tile([CC, 1], f32, tag="r12")
            conu_sb = pp.tile([CC, 1], f32, tag="conu")
            y_sb = pp.tile([CC, 1], f32, tag="y")
            yt_sb = pp.tile([1, CC], f32, tag="yt")
            escr = pp.tile([1, 1], f32, tag="escr")

            # ---- input DMAs: idx first (gathers wait on it); pb on the
            # scalar queue so both issue in parallel ----
            nc.sync.dma_start(out=idx_sb[:, :], in_=idx_d[:, :])
            nc.sync.dma_start(out=ps_sb[:, :], in_=ps_d[:, :])
            nc.scalar.dma_start(out=pb_sb[:, :], in_=pb_d[:, :])
            att_sb = pb_sb[:, 0:D]
            w_sb = pb_sb[:, D : 2 * D]
            ident = pb_sb[:, 2 * D : 3 * D]
            str1_sb = ps_sb[:, 0:DS]
            str2_sb = ps_sb[:, DS : 2 * DS]
            b_sb = ps_sb[:, 2 * DS : 2 * DS + 1]
            id32 = ps_sb[:, 2 * DS + 1 : 2 * DS + 1 + CC]

            # ---- constants ----
            nc.vector.memset(ones_bf[:, :], 1.0)
            nc.vector.memset(LT[:, :], 0.0)
            nc.vector.memset(magic_sb[:, :], RSQRT_MAGIC)

            # Preload the {Tanh, Exp} activation table set while gathers run
            # (the first ACT on a fresh set costs ~2.7us of table load).
            nc.scalar.activation(escr[:, :], ones_bf[0:1, 0:1], AF.Exp)

            # ---- gathers: A block first (AMT feeds everything), then B ----
            def gather(j):
                nc.gpsimd.indirect_dma_start(
                    out=BGt[j][:, :],
                    out_offset=None,
                    in_=table_d[:, :],
                    in_offset=bass.IndirectOffsetOnAxis(
                        ap=idx_sb[:, j : j + 1], axis=0
                    ),
                )

            gather(NP)
            for j in range(NP):
                gather(j)

            # ---- string branch: cosine similarity, all on DVE ----
            nc.vector.tensor_tensor(out=scr200[:, :], in0=str2_sb,
                                    in1=str1_sb, op=ALU.mult)
            nc.vector.reduce_sum(dot_sb[:, :], scr200[:, :], axis=AX.X)
            nc.vector.tensor_tensor(out=scr200[:, :], in0=str2_sb,
                                    in1=str2_sb, op=ALU.mult)
            nc.vector.reduce_sum(ssq2_sb[:, :], scr200[:, :], axis=AX.X)
            nc.vector.tensor_tensor(out=scr200[:, :], in0=str1_sb,
                                    in1=str1_sb, op=ALU.mult)
            nc.vector.reduce_sum(ssq1_sb[:, :], scr200[:, :], axis=AX.X)
            nc.vector.tensor_tensor(out=den2_sb[:, :], in0=ssq1_sb[:, :],
                                    in1=ssq2_sb[:, :], op=ALU.mult)
            # rsqrt(den2) via quake bit-hack + 2 Newton iterations
            rs_i = rs_sb[:, :].bitcast(i32)
            nc.vector.tensor_scalar(out=rs_i, in0=den2_sb[:, :].bitcast(i32),
                                    scalar1=1, scalar2=None,
                                    op0=ALU.logical_shift_right)
            nc.vector.tensor_tensor(out=rs_i, in0=magic_sb[:, :], in1=rs_i,
                                    op=ALU.subtract)
            for _ in range(2):
                nc.vector.tensor_tensor(out=nt_sb[:, :], in0=rs_sb[:, :],
                                        in1=rs_sb[:, :], op=ALU.mult)
                nc.vector.tensor_tensor(out=nt_sb[:, :], in0=nt_sb[:, :],
                                        in1=den2_sb[:, :], op=ALU.mult)
                nc.vector.tensor_scalar(out=nt_sb[:, :], in0=nt_sb[:, :],
                                        scalar1=-0.5, scalar2=1.5,
                                        op0=ALU.mult, op1=ALU.add)
                nc.vector.tensor_tensor(out=rs_sb[:, :], in0=rs_sb[:, :],
                                        in1=nt_sb[:, :], op=ALU.mult)
            nc.vector.tensor_tensor(out=strs_sb[:, :], in0=dot_sb[:, :],
                                    in1=rs_sb[:, :], op=ALU.mult)
            # sbh = 0.5*(str + b), computed while gathers still run
            nc.vector.tensor_scalar(out=sbh_sb[:, :], in0=strs_sb[:, :],
                                    scalar1=b_sb[:, 0:1], scalar2=GAMMA,
                                    op0=ALU.add, op1=ALU.mult)

            A_sb = BGt[NP][0:64, :]  # [K, D] gathered t1 rows, bf16

            # ---- A^T on PE; AMT = (A @ att).T; AW = A @ W ----
            A_T_p = ps_sm.tile([128, K], bf16, tag="sm")
            nc.tensor.transpose(A_T_p[:, :], A_sb, ident[0:64, 0:64])
            nc.scalar.copy(A_T_sb[:, :], A_T_p[:, :])
            AMT_p = ps_sm.tile([128, K], f32, tag="sm")
            nc.tensor.matmul(AMT_p[:, :], lhsT=att_sb, rhs=A_T_sb[:, :],
                             start=True, stop=True)
            nc.scalar.copy(AMT_sb[:, :], AMT_p[:, :])
            AW_p = ps_sm.tile([K, 128], f32, tag="sm")
            nc.tensor.matmul(AW_p[:, :], lhsT=A_T_sb[:, :], rhs=w_sb,
                             start=True, stop=True)
            nc.vector.tensor_copy(AW_sb[:, :], AW_p[:, :])

            # ---- main loop: descending chunk widths so the dependency
            # chain after the final gather stays short; stage A (transposes
            # + sim matmul) of chunk q is emitted before stage B (softmax /
            # newB / scores) of chunk q-1 so the in-order PE queue keeps
            # chasing the gather stream while scalar/DVE work on q-1. Scores
            # (s1, s2, z, y) are per-candidate, so each chunk finishes its
            # own y columns and only the last block's chain trails the
            # final gather. ----
            CHUNKS = [4, 4, 4, 3, 1]
            starts = [sum(CHUNKS[:i]) for i in range(len(CHUNKS))]
            stash = {}

            def stage_a(q):
                w = CHUNKS[q]
                t0 = starts[q]
                BT_chunk = btp.tile([128, 4 * D], bf16, tag="bts",
                                    name="bt_chunk")
                for i in range(w):
                    BT_p = ps_bt.tile([128, D], bf16, tag="btp", name="bt_p")
                    nc.tensor.transpose(BT_p[:, :], BGt[t0 + i][:, :],
                                        ident)
                    nc.vector.tensor_copy(
                        BT_chunk[:, D * i : D * (i + 1)], BT_p[:, :])
                sim_p = ps_simk.tile([K, 4 * D], f32, tag="simk",
                                     name="sim_p")
                nc.tensor.matmul(sim_p[:, 0 : w * D], lhsT=AMT_sb[:, :],
                                 rhs=BT_chunk[:, 0 : w * D],
                                 start=True, stop=True)
                c0 = 2 * t0
                # tanh + rows reduce live in stage A so a later chunk's tanh
                # is never head-blocked behind this chunk's exps on the
                # in-order scalar queue
                tanh_sb = tkp.tile([K, 4 * D], bf16, tag="tanhk",
                                   name="tanh_sb")
                nc.scalar.activation(tanh_sb[:, 0 : w * D],
                                     sim_p[:, 0 : w * D], AF.Tanh)
                nc.vector.reduce_sum(
                    R_T[:, c0 : c0 + 2 * w],
                    tanh_sb[:, 0 : w * D].rearrange("p (c m) -> p c m", m=K),
                    axis=AX.X,
                )
                stash[q] = tanh_sb

            def stage_b(q):
                w = CHUNKS[q]
                t0 = starts[q]
                c0 = 2 * t0
                w2 = 2 * w
                tanh_sb = stash.pop(q)
                nc.scalar.activation(ET2[:, c0 : c0 + w2],
                                     R_T[:, c0 : c0 + w2],
                                     AF.Exp, scale=1.0 / K)
                # one PSUM tile per chunk: cols [0,w) csum, [w,3w) newB^T,
                # [3w,5w) T1uT (PSUM bufs are bank-granular; packing keeps
                # the whole kernel within the 8 banks)
                ck_p = ps_ck.tile([128, 24], f32, tag="ck", name="ck_p")
                for i in range(w):
                    nc.tensor.matmul(
                        ck_p[:, i : i + 1],
                        lhsT=tanh_sb[:, D * i : D * (i + 1)],
                        rhs=ones_bf[0:K, :],
                        start=True, stop=True,
                    )
                # exp straight into the LT checkerboard (even candidates on
                # partitions 0-63, odd on 64-127)
                nc.scalar.activation(LT[0:64, c0 : c0 + w2 - 1 : 2],
                                     ck_p[0:64, 0:w], AF.Exp, scale=1.0 / K)
                nc.scalar.activation(LT[64:128, c0 + 1 : c0 + w2 : 2],
                                     ck_p[64:128, 0:w], AF.Exp, scale=1.0 / K)
                # newB^T for this chunk into column slices
                for i in range(w):
                    t = t0 + i
                    nc.tensor.matmul(
                        ck_p[:, w + 2 * i : w + 2 * i + 2],
                        lhsT=BGt[t][:, :],
                        rhs=LT[:, 2 * t : 2 * t + 2],
                        start=True, stop=True,
                    )
                last = q == len(CHUNKS) - 1
                if last:
                    # softmax denominators can start as soon as LT/ET2 are
                    # final -- before the newB/PZ part of the tail chain
                    sc_p = ps_sm.tile([CC, 2], f32, tag="sm")
                    nc.tensor.matmul(sc_p[:, 0:1], lhsT=ET2[:, :],
                                     rhs=ones_bf[0:K, :],
                                     start=True, stop=True)
                    nc.tensor.matmul(sc_p[:, 1:2], lhsT=LT[:, :],
                                     rhs=ones_bf[:, :], start=True, stop=True)
                if last:
                    # own PSUM tile (the shared ck_p tile would serialize
                    # this matmul behind the VBT copy) and an early SBUF
                    # copy: T1uT only needs ET2, so it is ready ~1us before
                    # newB and the PZ multiply can then read newB straight
                    # from PSUM -- one hop less on the post-gather tail
                    t1_p = ps_sm.tile([128, 2], f32, tag="sm")
                    nc.tensor.matmul(t1_p[:, :], lhsT=AW_sb[:, :],
                                     rhs=ET2[:, c0 : c0 + w2],
                                     start=True, stop=True)
                    nc.vector.tensor_copy(VBT_sb[:, c0 : c0 + w2],
                                          t1_p[:, :])
                else:
                    nc.tensor.matmul(ck_p[:, 3 * w : 3 * w + w2],
                                     lhsT=AW_sb[:, :],
                                     rhs=ET2[:, c0 : c0 + w2],
                                     start=True, stop=True)
                    nc.vector.tensor_copy(VBT_sb[:, c0 : c0 + w2],
                                          ck_p[:, w : w + w2])
                if last:
                    # r12 = 1/(2*s1*s2) so y folds into one tensor_scalar
                    nc.vector.tensor_copy(s2_sb2[:, :], sc_p[:, 1:2])
                    nc.vector.tensor_scalar(out=s12_sb[:, :],
                                            in0=sc_p[:, 0:1],
                                            scalar1=s2_sb2[:, 0:1],
                                            scalar2=1.0 / GAMMA,
                                            op0=ALU.mult, op1=ALU.mult)
                    nc.vector.reciprocal(r12_sb[:, :], s12_sb[:, :])
                # PZ = T1uT * VBT chunk; for the last chunk VBT_sb holds
                # T1uT (copied early) and newB is read from PSUM
                nc.vector.tensor_tensor(out=PZ_sb[:, c0 : c0 + w2],
                                        in0=(ck_p[:, w : w + w2] if last
                                             else ck_p[:, 3 * w : 3 * w + w2]),
                                        in1=VBT_sb[:, c0 : c0 + w2],
                                        op=ALU.mult)
                if last:
                    z_p = ps_sm.tile([CC, 1], f32, tag="sm")
                    nc.tensor.matmul(z_p[:, :], lhsT=PZ_sb[:, :],
                                     rhs=ones_bf[:, :], start=True, stop=True)
                    nc.vector.tensor_scalar(out=y_sb[:, :], in0=z_p[:, :],
                                            scalar1=r12_sb[:, 0:1],
                                            scalar2=sbh_sb[:, 0:1],
                                            op0=ALU.mult, op1=ALU.add)

            stage_a(0)
            for q in range(1, len(CHUNKS)):
                stage_a(q)
                stage_b(q - 1)
            stage_b(len(CHUNKS) - 1)

            # y as [1, CC]: a [CC,1] source sprays into 32 tiny per-partition
            # descriptors (~2us extra DMA latency); one PE transpose makes
            # the store a single contiguous 128B descriptor
            y_p = ps_sm.tile([1, CC], f32, tag="sm")
            nc.tensor.transpose(y_p[:, :], y_sb[:, :], id32)
            nc.vector.tensor_copy(yt_sb[:, :], y_p[:, :])
            nc.sync.dma_start(out=y_d[:, :], in_=yt_sb[:, :])

    nc.compile()
    return nc


def get_nc():
    global _BUILT
    if _BUILT is None:
        _BUILT = _build_nc()
    return _BUILT


def make_in_maps(table, str_t1, str_t2s, att_mat, W_bi, b_bi, t1_ctx, t2_ctx):
    import ml_dtypes

    bf16 = ml_dtypes.bfloat16
    table_bf = np.ascontiguousarray(
        np.asarray(table, dtype=np.float32).astype(bf16)
    )
    str_t1 = np.asarray(str_t1, dtype=np.float32).reshape(DS)
    str_t2s = np.asarray(str_t2s, dtype=np.float32)
    att_bf = np.asarray(att_mat, dtype=np.float32).astype(bf16)
    w_bf = np.asarray(W_bi, dtype=np.float32).reshape(D, D).astype(bf16)
    bval = float(np.asarray(b_bi).reshape(-1)[0])
    t1 = np.asarray(t1_ctx).astype(np.int32)
    t2 = np.asarray(t2_ctx).astype(np.int32)

    pb = np.empty((128, 3 * D), bf16)
    pb[:, 0:D] = att_bf
    pb[:, D : 2 * D] = w_bf
    pb[:, 2 * D : 3 * D] = np.eye(D, dtype=np.float32).astype(bf16)

    in_maps = []
    for i in range(NCORES):
        c0 = i * CC
        t2s = t2[c0 : c0 + CC]  # [CC, K]
        idx = np.empty((128, NB), np.int32)
        idx[0:64, 0:NP] = t2s[0::2, :].T    # even candidates, partitions 0-63
        idx[64:128, 0:NP] = t2s[1::2, :].T  # odd candidates, partitions 64-127
        idx[0:64, NP] = t1
        idx[64:128, NP] = t1
        ps = np.empty((CC, 2 * DS + 1 + CC), np.float32)
        ps[:, 0:DS] = str_t1
        ps[:, DS : 2 * DS] = str_t2s[c0 : c0 + CC]
        ps[:, 2 * DS] = bval
        ps[:, 2 * DS + 1 :] = np.eye(CC, dtype=np.float32)
        in_maps.append({
            "table": table_bf,
            "idx": idx,
            "pb": pb,
            "ps": ps,
        })
    return in_maps


def run(inputs: dict, trace: bool = False):
    from concourse.bass_utils import run_bass_kernel_spmd

    nc = get_nc()
    in_maps = make_in_maps(**inputs)
    res = run_bass_kernel_spmd(
        nc, in_maps, core_ids=list(range(NCORES)), trace=trace
    )
    y = np.concatenate([r["y"].reshape(-1) for r in res.results])
    return y.reshape(1, C).astype(np.float32), res


def kernel(**inputs) -> np.ndarray:
    y, _ = run(inputs, trace=False)
    return y


# revision 26
# speedup vs baseline: 1.0098x; 1.0098x over previous
"""DeepTermRankingListNet Trainium2 kernel.

Full-input contract: kernel(**inputs) takes the unsharded numpy inputs and
returns the full [1, 256] output. Internally shards candidates C=256 across
8 NeuronCores (32 each), replicates the embedding table + small params,
runs one SPMD Bass/Tile kernel via run_bass_kernel_spmd, and concatenates
the per-core [32] outputs.

Per-core device program (CC=32 candidates, K=64 ctx rows, d=128):
  1. The embedding table is shipped to DRAM as bf16 (rel-err budget 2e-2;
     measured end-to-end error of the bf16 pipeline is ~3e-3, and the
     gathers move half the bytes). 17 indirect DMAs (one index per
     partition each -- the HW SWDGE ucode generates exactly one
     descriptor per offset-AP partition, so 128 rows per instruction is
     the hard limit) gather the 32*64 t2-ctx rows + 64 t1-ctx rows.
     The ~1.33us/instruction GpSimd issue cost dominates the kernel; all
     compute is arranged to chase the gather stream.
  2. Per pair-block: one bf16 PE transpose (single pass vs 2 for f32)
     into PSUM, copied to the chunk's BT tile.
  3. Per chunk of 4 pair-blocks: one k-major sim matmul with stationary
     AMT (bf16); tanh (bf16 out); rows-softmax numerators via a free-axis
     reduce; cols-softmax numerators via ones-vector matmuls on the tanh
     blocks (partition reduce on the PE, replacing the m-major sim
     matmuls + second tanh of the f32 version); exp writes straight into
     the LT checkerboard / ET2; newB^T per pair into PSUM column slices;
     bilinear partials (T1uT = (A@W)^T E chunk, PZ = T1uT * VBT chunk).
     Softmax max-subtraction is skipped: tanh output is in [-1,1] so exp
     never overflows.
  4. Softmax denominators via ones matmuls; z = ones-reduce of PZ;
     con = z/(s1*s2).
  5. Cosine-similarity string branch on [32, 200] f32 tiles entirely on
     DVE, with rsqrt via the quake bit-hack + 2 Newton steps so the
     scalar engine's activation-table set stays {Tanh, Exp} (a Sqrt
     would force two extra ~1.5us table loads + drains mid-kernel).
"""

import numpy as np

V, D, K, C, DS = 500000, 128, 64, 256, 200
NCORES = 8
CC = C // NCORES  # 32 candidates per core
NP = CC // 2      # 16 candidate-pair blocks
NB = NP + 1       # + 1 block for A (t1_ctx rows)
GAMMA = 0.5
RSQRT_MAGIC = 0x5F3759DF

_BUILT = None


def _build_nc():
    import concourse.bacc as bacc
    import concourse.mybir as mybir
    from concourse import bass
    from concourse.tile import TileContext

    f32 = mybir.dt.float32
    bf16 = mybir.dt.bfloat16
    i32 = mybir.dt.int32
    AF = mybir.ActivationFunctionType
    ALU = mybir.AluOpType
    AX = mybir.AxisListType

    nc = bacc.Bacc("TRN2", debug=False)

    table_d = nc.dram_tensor("table", (V, D), bf16, kind="ExternalInput")
    idx_d = nc.dram_tensor("idx", (128, NB), i32, kind="ExternalInput")
    # packed bf16 params: att | w | identity
    pb_d = nc.dram_tensor("pb", (128, 3 * D), bf16, kind="ExternalInput")
    # packed f32 params: str1(broadcast) | str2 | b
    ps_d = nc.dram_tensor("ps", (CC, 2 * DS + 1 + CC), f32, kind="ExternalInput")
    y_d = nc.dram_tensor("y", (1, CC), f32, kind="ExternalOutput")

    with TileContext(nc) as tc:
        with (
            tc.tile_pool(name="pers", bufs=1) as pp,
            tc.tile_pool(name="bts", bufs=2) as btp,
            tc.tile_pool(name="tanhk", bufs=2) as tkp,
            tc.tile_pool(name="ps_bt", bufs=2, space="PSUM") as ps_bt,
            tc.tile_pool(name="ps_simk", bufs=2, space="PSUM") as ps_simk,
            tc.tile_pool(name="ps_ck", bufs=2, space="PSUM") as ps_ck,
            tc.tile_pool(name="ps_sm", bufs=2, space="PSUM") as ps_sm,
        ):
            # ---- persistent SBUF tiles ----
            idx_sb = pp.tile([128, NB], i32, tag="idx")
            BGt = [
                pp.tile([128, D], bf16, tag=f"bg{j}", name=f"bg{j}")
                for j in range(NB)
            ]
            pb_sb = pp.tile([128, 3 * D], bf16, tag="pb")
            ps_sb = pp.tile([CC, 2 * DS + 1 + CC], f32, tag="ps")
            ones_bf = pp.tile([128, 1], bf16, tag="ones")
            A_T_sb = pp.tile([128, K], bf16, tag="at")
            AMT_sb = pp.tile([128, K], bf16, tag="amt")
            AW_sb = pp.tile([K, 128], bf16, tag="aw")
            LT = pp.tile([128, CC], bf16, tag="lt")
            R_T = pp.tile([K, CC], f32, tag="rt")
            ET2 = pp.tile([K, CC], bf16, tag="et2")
            VBT_sb = pp.tile([128, CC], f32, tag="vbt")
            PZ_sb = pp.tile([128, CC], bf16, tag="pz")
            scr200 = pp.tile([CC, DS], f32, tag="scr200")
            # small [CC,1] scalars (string branch + final combine)
            magic_sb = pp.tile([CC, 1], i32, tag="magic")
            dot_sb = pp.tile([CC, 1], f32, tag="dot")
            ssq2_sb = pp.tile([CC, 1], f32, tag="ssq2")
            ssq1_sb = pp.tile([CC, 1], f32, tag="ssq1")
            den2_sb = pp.tile([CC, 1], f32, tag="den2")
            rs_sb = pp.tile([CC, 1], f32, tag="rs")
            nt_sb = pp.tile([CC, 1], f32, tag="nt")
            strs_sb = pp.tile([CC, 1], f32, tag="strs")
            sbh_sb = pp.tile([CC, 1], f32, tag="sbh")
            s12_sb = pp.tile([CC, 1], f32, tag="s12")
            s2_sb2 = pp.tile([CC, 1], f32, tag="s2c")
            r12_sb = pp.tile([CC, 1], f32, tag="r12")
            conu_sb = pp.tile([CC, 1], f32, tag="conu")
            y_sb = pp.tile([CC, 1], f32, tag="y")
            yt_sb = pp.tile([1, CC], f32, tag="yt")
            escr = pp.tile([1, 1], f32, tag="escr")

            # ---- input DMAs: idx first (gathers wait on it); pb on the
            # scalar queue so both issue in parallel ----
            nc.sync.dma_start(out=idx_sb[:, :], in_=idx_d[:, :])
            nc.scalar.dma_start(out=pb_sb[:, :], in_=pb_d[:, :])
            nc.scalar.dma_start(out=ps_sb[:, :], in_=ps_d[:, :])
            att_sb = pb_sb[:, 0:D]
            w_sb = pb_sb[:, D : 2 * D]
            ident = pb_sb[:, 2 * D : 3 * D]
            str1_sb = ps_sb[:, 0:DS]
            str2_sb = ps_sb[:, DS : 2 * DS]
            b_sb = ps_sb[:, 2 * DS : 2 * DS + 1]
            id32 = ps_sb[:, 2 * DS + 1 : 2 * DS + 1 + CC]

            # ---- constants ----
            nc.vector.memset(ones_bf[:, :], 1.0)
            nc.vector.memset(LT[:, :], 0.0)
            nc.vector.memset(magic_sb[:, :], RSQRT_MAGIC)

            # Preload the {Tanh, Exp} activation table set while gathers run
            # (the first ACT on a fresh set costs ~2.7us of table load).
            nc.scalar.activation(escr[:, :], ones_bf[0:1, 0:1], AF.Exp)

            # ---- gathers: A block first (AMT feeds everything), then B ----
            def gather(j):
                nc.gpsimd.indirect_dma_start(
                    out=BGt[j][:, :],
                    out_offset=None,
                    in_=table_d[:, :],
                    in_offset=bass.IndirectOffsetOnAxis(
                        ap=idx_sb[:, j : j + 1], axis=0
                    ),
                )

            gather(NP)
            for j in range(NP):
                gather(j)

            # ---- string branch: cosine similarity, all on DVE ----
            nc.vector.tensor_tensor(out=scr200[:, :], in0=str2_sb,
                                    in1=str1_sb, op=ALU.mult)
            nc.vector.reduce_sum(dot_sb[:, :], scr200[:, :], axis=AX.X)
            nc.vector.tensor_tensor(out=scr200[:, :], in0=str2_sb,
                                    in1=str2_sb, op=ALU.mult)
            nc.vector.reduce_sum(ssq2_sb[:, :], scr200[:, :], axis=AX.X)
            nc.vector.tensor_tensor(out=scr200[:, :], in0=str1_sb,
                                    in1=str1_sb, op=ALU.mult)
            nc.vector.reduce_sum(ssq1_sb[:, :], scr200[:, :], axis=AX.X)
            nc.vector.tensor_tensor(out=den2_sb[:, :], in0=ssq1_sb[:, :],
                                    in1=ssq2_sb[:, :], op=ALU.mult)
            # rsqrt(den2) via quake bit-hack + 2 Newton iterations
            rs_i = rs_sb[:, :].bitcast(i32)
            nc.vector.tensor_scalar(out=rs_i, in0=den2_sb[:, :].bitcast(i32),
                                    scalar1=1, scalar2=None,
                                    op0=ALU.logical_shift_right)
            nc.vector.tensor_tensor(out=rs_i, in0=magic_sb[:, :], in1=rs_i,
                                    op=ALU.subtract)
            for _ in range(2):
                nc.vector.tensor_tensor(out=nt_sb[:, :], in0=rs_sb[:, :],
                                        in1=rs_sb[:, :], op=ALU.mult)
                nc.vector.tensor_tensor(out=nt_sb[:, :], in0=nt_sb[:, :],
                                        in1=den2_sb[:, :], op=ALU.mult)
                nc.vector.tensor_scalar(out=nt_sb[:, :], in0=nt_sb[:, :],
                                        scalar1=-0.5, scalar2=1.5,
                                        op0=ALU.mult, op1=ALU.add)
                nc.vector.tensor_tensor(out=rs_sb[:, :], in0=rs_sb[:, :],
                                        in1=nt_sb[:, :], op=ALU.mult)
            nc.vector.tensor_tensor(out=strs_sb[:, :], in0=dot_sb[:, :],
                                    in1=rs_sb[:, :], op=ALU.mult)
            # sbh = 0.5*(str + b), computed while gathers still run
            nc.vector.tensor_scalar(out=sbh_sb[:, :], in0=strs_sb[:, :],
                                    scalar1=b_sb[:, 0:1], scalar2=GAMMA,
                                    op0=ALU.add, op1=ALU.mult)

            A_sb = BGt[NP][0:64, :]  # [K, D] gathered t1 rows, bf16

            # ---- A^T on PE; AMT = (A @ att).T; AW = A @ W ----
            A_T_p = ps_sm.tile([128, K], bf16, tag="sm")
            nc.tensor.transpose(A_T_p[:, :], A_sb, ident[0:64, 0:64])
            nc.scalar.copy(A_T_sb[:, :], A_T_p[:, :])
            AMT_p = ps_sm.tile([128, K], f32, tag="sm")
            nc.tensor.matmul(AMT_p[:, :], lhsT=att_sb, rhs=A_T_sb[:, :],
                             start=True, stop=True)
            nc.scalar.copy(AMT_sb[:, :], AMT_p[:, :])
            AW_p = ps_sm.tile([K, 128], f32, tag="sm")
            nc.tensor.matmul(AW_p[:, :], lhsT=A_T_sb[:, :], rhs=w_sb,
                             start=True, stop=True)
            nc.vector.tensor_copy(AW_sb[:, :], AW_p[:, :])

            # ---- main loop: descending chunk widths so the dependency
            # chain after the final gather stays short; stage A (transposes
            # + sim matmul) of chunk q is emitted before stage B (softmax /
            # newB / scores) of chunk q-1 so the in-order PE queue keeps
            # chasing the gather stream while scalar/DVE work on q-1. Scores
            # (s1, s2, z, y) are per-candidate, so each chunk finishes its
            # own y columns and only the last block's chain trails the
            # final gather. ----
            CHUNKS = [4, 4, 4, 3, 1]
            starts = [sum(CHUNKS[:i]) for i in range(len(CHUNKS))]
            stash = {}

            def stage_a(q):
                w = CHUNKS[q]
                t0 = starts[q]
                BT_chunk = btp.tile([128, 4 * D], bf16, tag="bts",
                                    name="bt_chunk")
                for i in range(w):
                    BT_p = ps_bt.tile([128, D], bf16, tag="btp", name="bt_p")
                    nc.tensor.transpose(BT_p[:, :], BGt[t0 + i][:, :],
                                        ident)
                    nc.vector.tensor_copy(
                        BT_chunk[:, D * i : D * (i + 1)], BT_p[:, :])
                sim_p = ps_simk.tile([K, 4 * D], f32, tag="simk",
                                     name="sim_p")
                nc.tensor.matmul(sim_p[:, 0 : w * D], lhsT=AMT_sb[:, :],
                                 rhs=BT_chunk[:, 0 : w * D],
                                 start=True, stop=True)
                c0 = 2 * t0
                # tanh + rows reduce live in stage A so a later chunk's tanh
                # is never head-blocked behind this chunk's exps on the
                # in-order scalar queue
                tanh_sb = tkp.tile([K, 4 * D], bf16, tag="tanhk",
                                   name="tanh_sb")
                nc.scalar.activation(tanh_sb[:, 0 : w * D],
                                     sim_p[:, 0 : w * D], AF.Tanh)
                nc.vector.reduce_sum(
                    R_T[:, c0 : c0 + 2 * w],
                    tanh_sb[:, 0 : w * D].rearrange("p (c m) -> p c m", m=K),
                    axis=AX.X,
                )
                stash[q] = tanh_sb

            def stage_b(q):
                w = CHUNKS[q]
                t0 = starts[q]
                c0 = 2 * t0
                w2 = 2 * w
                tanh_sb = stash.pop(q)
                nc.scalar.activation(ET2[:, c0 : c0 + w2],
                                     R_T[:, c0 : c0 + w2],
                                     AF.Exp, scale=1.0 / K)
                # one PSUM tile per chunk: cols [0,w) csum, [w,3w) newB^T,
                # [3w,5w) T1uT (PSUM bufs are bank-granular; packing keeps
                # the whole kernel within the 8 banks)
                ck_p = ps_ck.tile([128, 24], f32, tag="ck", name="ck_p")
                for i in range(w):
                    nc.tensor.matmul(
                        ck_p[:, i : i + 1],
                        lhsT=tanh_sb[:, D * i : D * (i + 1)],
                        rhs=ones_bf[0:K, :],
                        start=True, stop=True,
                    )
                # exp straight into the LT checkerboard (even candidates on
                # partitions 0-63, odd on 64-127)
                nc.scalar.activation(LT[0:64, c0 : c0 + w2 - 1 : 2],
                                     ck_p[0:64, 0:w], AF.Exp, scale=1.0 / K)
                nc.scalar.activation(LT[64:128, c0 + 1 : c0 + w2 : 2],
                                     ck_p[64:128, 0:w], AF.Exp, scale=1.0 / K)
                # newB^T for this chunk into column slices
                for i in range(w):
                    t = t0 + i
                    nc.tensor.matmul(
                        ck_p[:, w + 2 * i : w + 2 * i + 2],
                        lhsT=BGt[t][:, :],
                        rhs=LT[:, 2 * t : 2 * t + 2],
                        start=True, stop=True,
                    )
                last = q == len(CHUNKS) - 1
                if last:
                    # softmax denominators can start as soon as LT/ET2 are
                    # final -- before the newB/PZ part of the tail chain
                    sc_p = ps_sm.tile([CC, 2], f32, tag="sm")
                    nc.tensor.matmul(sc_p[:, 0:1], lhsT=ET2[:, :],
                                     rhs=ones_bf[0:K, :],
                                     start=True, stop=True)
                    nc.tensor.matmul(sc_p[:, 1:2], lhsT=LT[:, :],
                                     rhs=ones_bf[:, :], start=True, stop=True)
                if last:
                    # own PSUM tile (the shared ck_p tile would serialize
                    # this matmul behind the VBT copy) and an early SBUF
                    # copy: T1uT only needs ET2, so it is ready ~1us before
                    # newB and the PZ multiply can then read newB straight
                    # from PSUM -- one hop less on the post-gather tail
                    t1_p = ps_sm.tile([128, 2], f32, tag="sm")
                    nc.tensor.matmul(t1_p[:, :], lhsT=AW_sb[:, :],
                                     rhs=ET2[:, c0 : c0 + w2],
                                     start=True, stop=True)
                    nc.vector.tensor_copy(VBT_sb[:, c0 : c0 + w2],
                                          t1_p[:, :])
                else:
                    nc.tensor.matmul(ck_p[:, 3 * w : 3 * w + w2],
                                     lhsT=AW_sb[:, :],
                                     rhs=ET2[:, c0 : c0 + w2],
                                     start=True, stop=True)
                    nc.vector.tensor_copy(VBT_sb[:, c0 : c0 + w2],
                                          ck_p[:, w : w + w2])
                if last:
                    # r12 = 1/(2*s1*s2) so y folds into one tensor_scalar
                    nc.vector.tensor_copy(s2_sb2[:, :], sc_p[:, 1:2])
                    nc.vector.tensor_scalar(out=s12_sb[:, :],
                                            in0=sc_p[:, 0:1],
                                            scalar1=s2_sb2[:, 0:1],
                                            scalar2=1.0 / GAMMA,
                                            op0=ALU.mult, op1=ALU.mult)
                    nc.vector.reciprocal(r12_sb[:, :], s12_sb[:, :])
                # PZ = T1uT * VBT chunk; for the last chunk VBT_sb holds
                # T1uT (copied early) and newB is read from PSUM
                nc.vector.tensor_tensor(out=PZ_sb[:, c0 : c0 + w2],
                                        in0=(ck_p[:, w : w + w2] if last
                                             else ck_p[:, 3 * w : 3 * w + w2]),
                                        in1=VBT_sb[:, c0 : c0 + w2],
                                        op=ALU.mult)
                if last:
                    z_p = ps_sm.tile([CC, 1], f32, tag="sm")
                    nc.tensor.matmul(z_p[:, :], lhsT=PZ_sb[:, :],
                                     rhs=ones_bf[:, :], start=True, stop=True)
                    nc.vector.tensor_scalar(out=y_sb[:, :], in0=z_p[:, :],
                                            scalar1=r12_sb[:, 0:1],
                                            scalar2=sbh_sb[:, 0:1],
                                            op0=ALU.mult, op1=ALU.add)

            stage_a(0)
            for q in range(1, len(CHUNKS)):
                stage_a(q)
                stage_b(q - 1)
            stage_b(len(CHUNKS) - 1)

            # y as [1, CC]: a [CC,1] source sprays into 32 tiny per-partition
            # descriptors (~2us extra DMA latency); one PE transpose makes
            # the store a single contiguous 128B descriptor
            y_p = ps_sm.tile([1, CC], f32, tag="sm")
            nc.tensor.transpose(y_p[:, :], y_sb[:, :], id32)
            nc.vector.tensor_copy(yt_sb[:, :], y_p[:, :])
            nc.sync.dma_start(out=y_d[:, :], in_=yt_sb[:, :])

    nc.compile()
    return nc


def get_nc():
    global _BUILT
    if _BUILT is None:
        _BUILT = _build_nc()
    return _BUILT


def make_in_maps(table, str_t1, str_t2s, att_mat, W_bi, b_bi, t1_ctx, t2_ctx):
    import ml_dtypes

    bf16 = ml_dtypes.bfloat16
    table_bf = np.ascontiguousarray(
        np.asarray(table, dtype=np.float32).astype(bf16)
    )
    str_t1 = np.asarray(str_t1, dtype=np.float32).reshape(DS)
    str_t2s = np.asarray(str_t2s, dtype=np.float32)
    att_bf = np.asarray(att_mat, dtype=np.float32).astype(bf16)
    w_bf = np.asarray(W_bi, dtype=np.float32).reshape(D, D).astype(bf16)
    bval = float(np.asarray(b_bi).reshape(-1)[0])
    t1 = np.asarray(t1_ctx).astype(np.int32)
    t2 = np.asarray(t2_ctx).astype(np.int32)

    pb = np.empty((128, 3 * D), bf16)
    pb[:, 0:D] = att_bf
    pb[:, D : 2 * D] = w_bf
    pb[:, 2 * D : 3 * D] = np.eye(D, dtype=np.float32).astype(bf16)

    in_maps = []
    for i in range(NCORES):
        c0 = i * CC
        t2s = t2[c0 : c0 + CC]  # [CC, K]
        idx = np.empty((128, NB), np.int32)
        idx[0:64, 0:NP] = t2s[0::2, :].T    # even candidates, partitions 0-63
        idx[64:128, 0:NP] = t2s[1::2, :].T  # odd candidates, partitions 64-127
        idx[0:64, NP] = t1
        idx[64:128, NP] = t1
        ps = np.empty((CC, 2 * DS + 1 + CC), np.float32)
        ps[:, 0:DS] = str_t1
        ps[:, DS : 2 * DS] = str_t2s[c0 : c0 + CC]
        ps[:, 2 * DS] = bval
        ps[:, 2 * DS + 1 :] = np.eye(CC, dtype=np.float32)
        in_maps.append({
            "table": table_bf,
            "idx": idx,
            "pb": pb,
            "ps": ps,
        })
    return in_maps


def run(inputs: dict, trace: bool = False):
    from concourse.bass_utils import run_bass_kernel_spmd

    nc = get_nc()
    in_maps = make_in_maps(**inputs)
    res = run_bass_kernel_spmd(
        nc, in_maps, core_ids=list(range(NCORES)), trace=trace
    )
    y = np.concatenate([r["y"].reshape(-1) for r in res.results])
    return y.reshape(1, C).astype(np.float32), res


def kernel(**inputs) -> np.ndarray:
    y, _ = run(inputs, trace=False)
    return y
